# revision 44
# baseline (speedup 1.0000x reference)
"""BasicNCA (neural cellular automaton) Trainium2 kernel, 8-core SPMD.

Reference computation (per step, 32 steps):
  p  = depthwise3x3(s, [identity, sobel_x, sobel_y])   # (B, 3C, H, W)
  h  = relu(w1 @ p + b1)                               # (B, 64, H, W)
  d  = w2 @ h + b2                                     # (B, C, H, W)
  s += d * (mask < 0.5)

Implementation notes:
- The perception conv + first 1x1 conv fuse into one effective 3x3 conv with
  weights Weff[o, c, dy, dx]. Its 9 taps are computed by 5 fp8e4m3 DoubleRow
  matmuls per slab-pair chunk: each contracts two weight planes whose rhs
  reads the fp8 state at two free-dim offsets (an overlapping-window access
  pattern, K_virtual=112, M=128), costing 0.5 PE cycles/column instead of
  1.0. The largest-weight tap (the identity center) uses both planes of one
  matmul as hi/lo split-precision fp8 weights at the same offset, cutting
  its quantization error to ~0.2%. Weights are pre-scaled by WSCALE=128 to
  stay in fp8 normal range; the relu activation un-scales via its scale arg.
- An fp8 shadow of the state (s8) feeds the matmuls; DVE 2x copy-casts
  refresh the 6 updated rows of each chunk pair after its residual update.
  The fp32 state is authoritative; fp8 error does not accumulate.
- Sharding: core i handles batch i//2, H-half i%2, with a 32-row taper of
  redundant compute instead of per-step halo exchange between cores.
- A core's 96-row slab splits into 4 sub-slabs of 24 rows on the 4 SBUF
  partition quadrants; cross-quadrant halo rows of s8 are refreshed with
  DVE stream_shuffles (nch<=32 quadrant moves), keeping DMA latency out of
  the step-boundary dependency chain.
- Masks are host-converted to fire=(m<0.5) in bf16, channel-replicated, and
  streamed per step. Delta masking is scalar_tensor_tensor (d + b2) * fire
  on DVE (reading delta straight from PSUM); the fp32 state adds run on the
  Pool engine (its only walrus-supported binop, TensorTensor Add) except
  the step's last chunk pair, which stays on DVE.
- The second 1x1 conv runs in full 128x128 fp32r mode (K=128 over both
  slabs' h, M=120 with per-slab 24-column blocks).
- Chunk pairs run in constant order (0, 3, 1, 2): the next step's first
  matmul group depends only on pairs {0,1,3} and the early stream_shuffles,
  so pair 2's mask/add/cast boundary chain overlaps mm(0) of step t+1 and
  the PE pipeline never re-enters its slow p-state. The software pipeline
  is fully cyclic: each pair's second L2 half (which needs that pair's
  relu) and its whole tail are emitted one L1 group later — pair 2's carry
  across the step boundary — so no matmul ever heads the in-order PE queue
  before its dependency resolves. Within a group, chunk cc=1 (which never
  reads the refreshed halo row) is emitted first, giving the boundary
  stream_shuffles extra slack.
"""

import sys

sys.path.insert(0, "/opt/trn_rl_repo")

import numpy as np
import ml_dtypes

import bass_rust
import concourse.bass as bass
import concourse.bacc as bacc
import concourse.tile as tile
import concourse.mybir as mybir

dt = mybir.dt
F8 = ml_dtypes.float8_e4m3
BF16 = ml_dtypes.bfloat16

B, C, H, W = 4, 24, 128, 128
HID = 64
FIRE_RATE = 0.5
N_CORES = 8

SH = 96            # slab rows per core (64 own + 32 taper)
SR = 24            # rows per sub-slab (one partition quadrant)
FW = W + 2         # padded row width (130)
FR = SR + 2        # frame rows per sub-slab (26)
FRAME_OFF = 4      # leading guard elems so tap offset -1 stays in-bounds
FRAME = FR * FW    # 3380
S_FREE = FRAME_OFF + FRAME + 4
COMP = SR * FW     # 3120 compact free size (real rows 0..23)
NCH = 390          # chunk = 3 rows
NCHUNK = COMP // NCH  # 8
NPAIR = NCHUNK // 2   # chunk pairs per step

WSCALE = 128.0

# DoubleRow plane pairs: each fp8 matmul contracts two weight planes at rhs
# plane stride off(p1)-off(p0) (even, for hw alignment). 6 matmuls give 12
# plane slots for the 9 taps; the three largest-weight taps — the identity
# center (1,1) and the sobel-x centers (1,0), (1,2) — get hi/lo
# split-precision fp8 weights (both planes read the same offset, stride 0),
# cutting their weight quantization error from ~3% to ~0.2%.
TAP_PAIRS = [
    (((1, 1), "hi"), ((1, 1), "lo")),   # delta = 0; no top-halo read
    (((2, 0), "hi"), ((2, 2), "hi")),   # delta = 2; no top-halo read
    (((0, 0), "hi"), ((1, 0), "hi")),   # delta = FW
    (((0, 2), "hi"), ((1, 2), "hi")),   # delta = FW
    (((0, 1), "hi"), ((2, 1), "hi")),   # delta = 2*FW
]
NDR = len(TAP_PAIRS)

LAST_EXEC_NS = None
_cache = {}


def _tap_off(dy, dx):
    return dy * FW + dx - 1


def _build_program(steps, repeats=1):
    nc = bacc.Bacc("TRN2", target_bir_lowering=False, debug=False,
                   num_devices=N_CORES)

    s_d = nc.dram_tensor("s0", [128, S_FREE], dt.float32r, kind="ExternalInput")
    m_d = nc.dram_tensor("fire", [steps, 128, COMP], dt.bfloat16,
                         kind="ExternalInput")
    tapw_d = nc.dram_tensor("tapw8", [128, NDR * 256], dt.float8e4,
                            kind="ExternalInput")
    w2b_d = nc.dram_tensor("w2b", [128, 2 * 120], dt.float32r,
                           kind="ExternalInput")
    b2r_d = nc.dram_tensor("b2r", [128, 1], dt.float32, kind="ExternalInput")
    b1_d = nc.dram_tensor("b1v", [128, 1], dt.float32, kind="ExternalInput")
    out_d = nc.dram_tensor("out", [128, SR * W], dt.float32,
                           kind="ExternalOutput")

    DR = mybir.MatmulPerfMode.DoubleRow
    Relu = mybir.ActivationFunctionType.Relu
    A = mybir.AluOpType

    def dr_rhs(s8, base, c, pair_i):
        ((dy0, dx0), _), ((dy1, dx1), _) = TAP_PAIRS[pair_i]
        off0 = FRAME_OFF + (3 * c + dy0) * FW + dx0 - 1
        delta = _tap_off(dy1, dx1) - _tap_off(dy0, dx0)
        rhs = s8[base:base + 56, off0:off0 + NCH].copy()
        rhs.ap = bass_rust.VecI64Pair(
            [[rhs.ap[0][0], 56], [delta, 2], [1, NCH]])
        return rhs

    with tile.TileContext(nc) as tc:
        with tc.tile_pool(name="persist", bufs=1) as pp, \
             tc.tile_pool(name="mpool", bufs=2) as mpool, \
             tc.tile_pool(name="dpool", bufs=2) as dpool, \
             tc.tile_pool(name="hsb", bufs=3) as hsbp, \
             tc.tile_pool(name="ps", bufs=4, space="PSUM") as ps_pool:

            s_sb = pp.tile([128, S_FREE], dt.float32r)
            s8 = pp.tile([128, S_FREE], dt.float8e4)
            tapw8 = pp.tile([128, NDR * 256], dt.float8e4)
            w2b = pp.tile([128, 2 * 120], dt.float32r)
            b2r = pp.tile([128, 1], dt.float32)
            b1v = pp.tile([128, 1], dt.float32)

            # state streamed in four pieces, each fp8-cast as it lands, so
            # the first matmuls start ~2us earlier; weights go after the
            # first piece (needed by the first matmul group too)
            _cuts = [0, FRAME_OFF + 8 * FW, FRAME_OFF + 14 * FW,
                     FRAME_OFF + 20 * FW, S_FREE]
            nc.sync.dma_start(s_sb[:, _cuts[0]:_cuts[1]],
                              s_d[:, _cuts[0]:_cuts[1]])
            nc.sync.dma_start(tapw8[:], tapw_d[:])
            nc.sync.dma_start(w2b[:], w2b_d[:])
            nc.sync.dma_start(b2r[:], b2r_d[:])
            nc.sync.dma_start(b1v[:], b1_d[:])
            for _i in range(1, 4):
                nc.sync.dma_start(s_sb[:, _cuts[_i]:_cuts[_i + 1]],
                                  s_d[:, _cuts[_i]:_cuts[_i + 1]])
            # initial fp8 shadow (includes halo rows prepped by the host)
            for _i in range(4):
                nc.vector.tensor_copy(s8[:, _cuts[_i]:_cuts[_i + 1]],
                                      s_sb[:, _cuts[_i]:_cuts[_i + 1]])

            a0 = FRAME_OFF + FW + 1

            def emit_out(k):
                nc.sync.dma_start(
                    out_d[:, 6 * k * W:(6 * k + 6) * W].rearrange(
                        "p (r x) -> p r x", x=W),
                    s_sb[:, a0 + 780 * k:a0 + 780 * k + 6 * FW].rearrange(
                        "p (r x) -> p r x", x=FW)[:, :, 0:W].bitcast(
                            dt.float32),
                )

            def emit_l1(k, p):
                base = 64 * p
                hps = ps_pool.tile([128, 1024], dt.float32, tag="ps",
                                    name="hps_t")
                for cc in (1, 0):
                    c = 2 * k + cc
                    for i in range(NDR):
                        nc.tensor.matmul(
                            hps[:, 512 * cc:512 * cc + NCH],
                            tapw8[base:base + 56,
                                  256 * i:256 * i + 256].rearrange(
                                      "p (two m) -> p two m", two=2),
                            dr_rhs(s8, base, c, i),
                            start=(i == 0), stop=(i == NDR - 1),
                            tile_position=(base, 0),
                            perf_mode=DR,
                        )
                hsb = hsbp.tile([128, 2 * NCH], dt.float32r, tag=f"hsb{p}",
                                name="hsb_t")
                nc.scalar.activation(
                    hsb[:].rearrange("p (b x) -> p b x", x=NCH),
                    hps[:].rearrange("p (b x) -> p b x", b=2)[:, :, 0:NCH],
                    Relu, bias=b1v[:, 0:1], scale=1.0 / WSCALE,
                )
                return hsb

            def emit_l2_half(dps, hsb, p):
                # one slab-pair half of the L2 accumulation for both chunks;
                # each half is emitted one L1 group after its relu so the
                # dependency never blocks the in-order PE queue
                for cc in range(2):
                    nc.tensor.matmul(
                        dps[0:120, 512 * cc:512 * cc + NCH],
                        w2b[:, 120 * p:120 * p + 120],
                        hsb[:, NCH * cc:NCH * cc + NCH],
                        start=(p == 0), stop=(p == 1),
                    )

            def emit_stt(k, dps, m_sb, d_sb):
                # u = (delta + b2) * fire on DVE (PSUM src), into d_sb
                nc.vector.scalar_tensor_tensor(
                    d_sb[0:120, 780 * k:780 * k + 780].rearrange(
                        "p (b x) -> p b x", x=NCH),
                    dps[0:120].rearrange(
                        "p (b x) -> p b x", b=2)[:, :, 0:NCH],
                    b2r[0:120, 0:1],
                    m_sb[0:120, 780 * k:780 * k + 780].rearrange(
                        "p (b x) -> p b x", x=NCH),
                    A.add, A.mult,
                )

            def emit_add(k, eng, d_sb):
                a = FRAME_OFF + FW + 780 * k
                # plain TensorTensor Add: the only elementwise binop the
                # walrus gpsimd codegen accepts (STT is DVE/ACT-only)
                eng.tensor_add(
                    s_sb[0:120, a:a + 780],
                    s_sb[0:120, a:a + 780],
                    d_sb[0:120, 780 * k:780 * k + 780],
                )

            def emit_cast(k):
                a = FRAME_OFF + FW + 780 * k
                nc.vector.tensor_copy(s8[:, a:a + 780], s_sb[:, a:a + 780])

            def emit_shuffles(edge):
                if edge == 0:
                    # row 24 of quadrant g-1 -> frame row 0 of quadrant g
                    for g in range(1, 4):
                        nc.vector.stream_shuffle(
                            s8[32 * g:32 * g + 32,
                               FRAME_OFF:FRAME_OFF + FW],
                            s8[32 * (g - 1):32 * (g - 1) + 32,
                               FRAME_OFF + 24 * FW:FRAME_OFF + 25 * FW],
                            list(range(32)))
                else:
                    # row 1 of quadrant g+1 -> frame row 25 of quadrant g
                    for g in range(3):
                        nc.vector.stream_shuffle(
                            s8[32 * g:32 * g + 32,
                               FRAME_OFF + 25 * FW:FRAME_OFF + 26 * FW],
                            s8[32 * (g + 1):32 * (g + 1) + 32,
                               FRAME_OFF + FW:FRAME_OFF + 2 * FW],
                            list(range(32)))

            def emit_carry(carry):
                # finish the previous pair: second L2 half, mask, state add.
                # For the boundary pair (2) this runs at the START of the
                # NEXT step's emission, overlapped by its first L1 group.
                ck, cdps, chsb1, cm, cd = carry
                emit_l2_half(cdps, chsb1, 1)
                emit_stt(ck, cdps, cm, cd)
                if ck == 2:
                    emit_add(ck, nc.vector, cd)
                    emit_cast(ck)
                else:
                    emit_add(ck, nc.gpsimd, cd)

            # Constant pair order with pair 2 last: the next step's first
            # group mm(0) depends only on pairs {0,1,3} (all early), so pair
            # 2's whole tail — including its final L2 half — carries across
            # the step boundary and overlaps mm(0, t+1).
            korder = [0, 3, 1, 2]
            carry = None
            for it in range(steps * repeats):
                t = it % steps
                last = it == steps * repeats - 1
                m_sb = mpool.tile([128, COMP], dt.bfloat16, tag="m")
                nc.sync.dma_start(m_sb[:], m_d[t])
                d_sb = dpool.tile([128, COMP], dt.float32, tag="d")

                for k in korder:
                    hsb0 = emit_l1(k, 0)
                    if carry is not None:
                        emit_carry(carry)
                    hsb1 = emit_l1(k, 1)
                    dps = ps_pool.tile([128, 1024], dt.float32, tag="ps",
                                        name="dps_t")
                    emit_l2_half(dps, hsb0, 0)
                    carry = (k, dps, hsb1, m_sb, d_sb)

                if last:
                    # no next step: flush the carry per chunk so the final
                    # writeback overlaps the mask/add chain, skipping all
                    # fp8-shadow maintenance
                    for kk in (0, 3, 1):
                        emit_out(kk)
                    ck, cdps, chsb1, cm, cd = carry
                    carry = None
                    emit_l2_half(cdps, chsb1, 1)
                    for cc in range(2):
                        nc.vector.scalar_tensor_tensor(
                            cd[0:120, 780 * ck + NCH * cc:
                               780 * ck + NCH * cc + NCH],
                            cdps[0:120, 512 * cc:512 * cc + NCH],
                            b2r[0:120, 0:1],
                            cm[0:120, 780 * ck + NCH * cc:
                               780 * ck + NCH * cc + NCH],
                            A.add, A.mult,
                        )
                        a = FRAME_OFF + FW + 780 * ck + NCH * cc
                        nc.vector.tensor_add(
                            s_sb[0:120, a:a + NCH],
                            s_sb[0:120, a:a + NCH],
                            cd[0:120, 780 * ck + NCH * cc:
                               780 * ck + NCH * cc + NCH],
                        )
                        nc.sync.dma_start(
                            out_d[:, (6 * ck + 3 * cc) * W:
                                  (6 * ck + 3 * cc + 3) * W].rearrange(
                                "p (r x) -> p r x", x=W),
                            s_sb[:, a0 + 780 * ck + NCH * cc:
                                 a0 + 780 * ck + NCH * cc + 3 * FW].rearrange(
                                "p (r x) -> p r x", x=FW)[:, :, 0:W].bitcast(
                                    dt.float32),
                        )
                else:
                    # s8 maintenance after the step's matmuls: later pairs'
                    # matmuls must read the pre-update boundary rows, and an
                    # early cast would stall them on a RAW hazard
                    emit_cast(3)
                    emit_shuffles(0)
                    emit_cast(0)
                    emit_cast(1)
                    emit_shuffles(25)

    nc.compile()
    return nc


def _prep_weights(w1, b1, w2, b2):
    sx = np.array([[-1, 0, 1], [-2, 0, 2], [-1, 0, 1]], np.float32) / 8.0
    sy = sx.T.copy()
    ident = np.zeros((3, 3), np.float32)
    ident[1, 1] = 1.0
    # Weff[o, c, dy, dx]
    weff = (np.einsum("oc,yx->ocyx", w1[:, 0::3], ident)
            + np.einsum("oc,yx->ocyx", w1[:, 1::3], sx)
            + np.einsum("oc,yx->ocyx", w1[:, 2::3], sy)).astype(np.float32)

    # DoubleRow pair lhsT: for pair-matmul i, [56 x 2 x 128] fp8 blocks at
    # partition bases 0 and 64 (one per slab pair). Rows 0-23 (even slab ch)
    # feed out cols 0-63; rows 32-55 (odd slab ch) feed cols 64-127.
    tapw8 = np.zeros((128, NDR * 256), F8)
    for i, pair in enumerate(TAP_PAIRS):
        for plane, ((dy, dx), level) in enumerate(pair):
            wt = weff[:, :, dy, dx].T * WSCALE   # [24, 64]
            w_hi = wt.astype(F8)
            if level == "hi":
                w8 = w_hi
            else:
                w8 = (wt - w_hi.astype(np.float32)).astype(F8)
            col = 256 * i + 128 * plane
            for pb in (0, 64):
                tapw8[pb:pb + 24, col:col + 64] = w8
                tapw8[pb + 32:pb + 56, col + 64:col + 128] = w8

    # layer2 lhsT per pair: K=128 (both h halves), M=120 with 24-col blocks
    # placing each slab's delta on its partition quadrant.
    w2b = np.zeros((128, 2 * 120), np.float32)
    for p in range(2):
        ge, go = 2 * p, 2 * p + 1
        w2b[0:64, 120 * p + 32 * ge:120 * p + 32 * ge + 24] = w2.T
        w2b[64:128, 120 * p + 32 * go:120 * p + 32 * go + 24] = w2.T

    b2r = np.zeros((128, 1), np.float32)
    b1v = np.zeros((128, 1), np.float32)
    for g in range(4):
        b2r[32 * g:32 * g + 24, 0] = b2
    b1v[0:64, 0] = b1
    b1v[64:128, 0] = b1
    return tapw8, w2b, b2r, b1v


def _prep_state(state):
    """state (B, C, H, W) -> per-core [128, S_FREE] framed slabs."""
    bufs = []
    for core in range(N_CORES):
        b = core // 2
        top = (core % 2) == 0
        r0 = 0 if top else H - SH
        buf = np.zeros((128, S_FREE), np.float32)
        for ch in range(C):
            full = np.zeros((SH + 2, FW), np.float32)
            full[1:SH + 1, 1:W + 1] = state[b, ch, r0:r0 + SH, :]
            if r0 > 0:
                full[0, 1:W + 1] = state[b, ch, r0 - 1, :]
            if r0 + SH < H:
                full[SH + 1, 1:W + 1] = state[b, ch, r0 + SH, :]
            for g in range(4):
                fr = full[g * SR:g * SR + FR, :]
                buf[32 * g + ch, FRAME_OFF:FRAME_OFF + FRAME] = fr.reshape(-1)
        bufs.append(buf)
    return bufs


def _prep_fire(masks):
    """masks (S, B, 1, H, W) -> per-core [S, 128, COMP] bf16 fire=(m<0.5),
    zero-padded so pad lanes/columns never update the state."""
    S = masks.shape[0]
    fire = (masks < FIRE_RATE).astype(np.float32)
    bufs = []
    for core in range(N_CORES):
        b = core // 2
        top = (core % 2) == 0
        r0 = 0 if top else H - SH
        mb = np.zeros((S, 128, COMP), np.float32)
        mrows = np.zeros((S, SH, FW), np.float32)
        mrows[:, :, 1:W + 1] = fire[:, b, 0, r0:r0 + SH, :]
        for g in range(4):
            seg = mrows[:, g * SR:(g + 1) * SR, :].reshape(S, COMP)
            mb[:, 32 * g:32 * g + C, :] = seg[:, None, :]
        bufs.append(mb.astype(BF16))
    return bufs


def prepare(inputs):
    """Build (nc, in_maps) for the given full inputs."""
    state = np.asarray(inputs["state"])
    w1, b1 = np.asarray(inputs["w1"]), np.asarray(inputs["b1"])
    w2, b2 = np.asarray(inputs["w2"]), np.asarray(inputs["b2"])
    masks = np.asarray(inputs["masks"])
    import os as _os
    steps = masks.shape[0]
    repeats = int(_os.environ.get("NCA_REPEAT", "1"))
    key = ("prog", steps, repeats)
    if key not in _cache:
        _cache[key] = _build_program(steps, repeats)
    nc = _cache[key]

    tapw8, w2b, b2r, b1v = _prep_weights(w1, b1, w2, b2)
    s_bufs = _prep_state(state)
    m_bufs = _prep_fire(masks)

    in_maps = []
    for core in range(N_CORES):
        in_maps.append({
            "s0": s_bufs[core],
            "fire": m_bufs[core],
            "tapw8": tapw8,
            "w2b": w2b,
            "b2r": b2r,
            "b1v": b1v,
        })
    return nc, in_maps


def unpack(results):
    out = np.zeros((B, C, H, W), np.float32)
    for core in range(N_CORES):
        o = results[core]["out"]  # [128, SR*W]
        b = core // 2
        top = (core % 2) == 0
        r0 = 0 if top else H - SH
        own0 = 0 if top else H // 2
        for g in range(4):
            rows = o[32 * g:32 * g + 24].reshape(C, SR, W)
            g0 = r0 + g * SR
            lo = max(g0, own0)
            hi = min(g0 + SR, own0 + H // 2)
            if lo < hi:
                out[b, :, lo:hi, :] = rows[:, lo - g0:hi - g0, :]
    return out


def kernel(state, w1, b1, w2, b2, masks):
    nc, in_maps = prepare({"state": state, "w1": w1, "b1": b1,
                           "w2": w2, "b2": b2, "masks": masks})

    from concourse.bass_utils import run_bass_kernel_spmd

    import os
    trace = bool(os.environ.get("NCA_TRACE"))
    kw = {}
    if trace:
        kw["trace"] = True
        if os.environ.get("NCA_TRACE_DIR"):
            kw["tmpdir"] = os.environ["NCA_TRACE_DIR"]
    res = run_bass_kernel_spmd(nc, in_maps, list(range(N_CORES)), **kw)
    global LAST_EXEC_NS
    LAST_EXEC_NS = res.exec_time_ns

    return unpack(res.results)


# revision 48
# speedup vs baseline: 1.0013x; 1.0013x over previous
"""BasicNCA (neural cellular automaton) Trainium2 kernel, 8-core SPMD.

Reference computation (per step, 32 steps):
  p  = depthwise3x3(s, [identity, sobel_x, sobel_y])   # (B, 3C, H, W)
  h  = relu(w1 @ p + b1)                               # (B, 64, H, W)
  d  = w2 @ h + b2                                     # (B, C, H, W)
  s += d * (mask < 0.5)

Implementation notes:
- The perception conv + first 1x1 conv fuse into one effective 3x3 conv with
  weights Weff[o, c, dy, dx]. Its 9 taps are computed by 5 fp8e4m3 DoubleRow
  matmuls per slab-pair chunk: each contracts two weight planes whose rhs
  reads the fp8 state at two free-dim offsets (an overlapping-window access
  pattern, K_virtual=112, M=128), costing 0.5 PE cycles/column instead of
  1.0. The largest-weight tap (the identity center) uses both planes of one
  matmul as hi/lo split-precision fp8 weights at the same offset, cutting
  its quantization error to ~0.2%. Weights are pre-scaled by WSCALE=128 to
  stay in fp8 normal range; the relu activation un-scales via its scale arg.
- An fp8 shadow of the state (s8) feeds the matmuls; DVE 2x copy-casts
  refresh the 6 updated rows of each chunk pair after its residual update.
  The fp32 state is authoritative; fp8 error does not accumulate.
- Sharding: core i handles batch i//2, H-half i%2, with a 32-row taper of
  redundant compute instead of per-step halo exchange between cores.
- A core's 96-row slab splits into 4 sub-slabs of 24 rows on the 4 SBUF
  partition quadrants; cross-quadrant halo rows of s8 are refreshed with
  DVE stream_shuffles (nch<=32 quadrant moves), keeping DMA latency out of
  the step-boundary dependency chain.
- Masks are host-converted to fire=(m<0.5) in bf16, channel-replicated, and
  streamed per step. Delta masking is scalar_tensor_tensor (d + b2) * fire
  on DVE (reading delta straight from PSUM); the fp32 state adds run on the
  Pool engine (its only walrus-supported binop, TensorTensor Add) except
  the step's last chunk pair, which stays on DVE.
- The second 1x1 conv runs in full 128x128 fp32r mode (K=128 over both
  slabs' h, M=120 with per-slab 24-column blocks).
- Chunk pairs run in constant order (0, 3, 1, 2): the next step's first
  matmul group depends only on pairs {0,1,3} and the early stream_shuffles,
  so pair 2's mask/add/cast boundary chain overlaps mm(0) of step t+1 and
  the PE pipeline never re-enters its slow p-state. The software pipeline
  is fully cyclic: each pair's second L2 half (which needs that pair's
  relu) and its whole tail are emitted one L1 group later — pair 2's carry
  across the step boundary — so no matmul ever heads the in-order PE queue
  before its dependency resolves. Within a group, chunk cc=1 (which never
  reads the refreshed halo row) is emitted first, giving the boundary
  stream_shuffles extra slack.
"""

import sys

sys.path.insert(0, "/opt/trn_rl_repo")

import numpy as np
import ml_dtypes

import bass_rust
import concourse.bass as bass
import concourse.bacc as bacc
import concourse.tile as tile
import concourse.mybir as mybir

dt = mybir.dt
F8 = ml_dtypes.float8_e4m3
BF16 = ml_dtypes.bfloat16

B, C, H, W = 4, 24, 128, 128
HID = 64
FIRE_RATE = 0.5
N_CORES = 8

SH = 96            # slab rows per core (64 own + 32 taper)
SR = 24            # rows per sub-slab (one partition quadrant)
FW = W + 2         # padded row width (130)
FR = SR + 2        # frame rows per sub-slab (26)
FRAME_OFF = 4      # leading guard elems so tap offset -1 stays in-bounds
FRAME = FR * FW    # 3380
S_FREE = FRAME_OFF + FRAME + 4
COMP = SR * FW     # 3120 compact free size (real rows 0..23)
NCH = 390          # chunk = 3 rows
NCHUNK = COMP // NCH  # 8
NPAIR = NCHUNK // 2   # chunk pairs per step

WSCALE = 128.0

# DoubleRow plane pairs: each fp8 matmul contracts two weight planes at rhs
# plane stride off(p1)-off(p0) (even, for hw alignment). 6 matmuls give 12
# plane slots for the 9 taps; the three largest-weight taps — the identity
# center (1,1) and the sobel-x centers (1,0), (1,2) — get hi/lo
# split-precision fp8 weights (both planes read the same offset, stride 0),
# cutting their weight quantization error from ~3% to ~0.2%.
TAP_PAIRS = [
    (((1, 1), "hi"), ((1, 1), "lo")),   # delta = 0; no top-halo read
    (((2, 0), "hi"), ((2, 2), "hi")),   # delta = 2; no top-halo read
    (((0, 0), "hi"), ((1, 0), "hi")),   # delta = FW
    (((0, 2), "hi"), ((1, 2), "hi")),   # delta = FW
    (((0, 1), "hi"), ((2, 1), "hi")),   # delta = 2*FW
]
NDR = len(TAP_PAIRS)

LAST_EXEC_NS = None
_cache = {}


def _tap_off(dy, dx):
    return dy * FW + dx - 1


def _build_program(steps, repeats=1):
    nc = bacc.Bacc("TRN2", target_bir_lowering=False, debug=False,
                   num_devices=N_CORES)

    s_d = nc.dram_tensor("s0", [128, S_FREE], dt.float32r, kind="ExternalInput")
    m_d = nc.dram_tensor("fire", [steps, 128, COMP], dt.bfloat16,
                         kind="ExternalInput")
    tapw_d = nc.dram_tensor("tapw8", [128, NDR * 256], dt.float8e4,
                            kind="ExternalInput")
    w2b_d = nc.dram_tensor("w2b", [128, 2 * 120], dt.float32r,
                           kind="ExternalInput")
    b2r_d = nc.dram_tensor("b2r", [128, 1], dt.float32, kind="ExternalInput")
    b1_d = nc.dram_tensor("b1v", [128, 1], dt.float32, kind="ExternalInput")
    out_d = nc.dram_tensor("out", [128, SR * W], dt.float32,
                           kind="ExternalOutput")

    DR = mybir.MatmulPerfMode.DoubleRow
    Relu = mybir.ActivationFunctionType.Relu
    A = mybir.AluOpType

    def dr_rhs(s8, base, c, pair_i):
        ((dy0, dx0), _), ((dy1, dx1), _) = TAP_PAIRS[pair_i]
        off0 = FRAME_OFF + (3 * c + dy0) * FW + dx0 - 1
        delta = _tap_off(dy1, dx1) - _tap_off(dy0, dx0)
        rhs = s8[base:base + 56, off0:off0 + NCH].copy()
        rhs.ap = bass_rust.VecI64Pair(
            [[rhs.ap[0][0], 56], [delta, 2], [1, NCH]])
        return rhs

    with tile.TileContext(nc) as tc:
        with tc.tile_pool(name="persist", bufs=1) as pp, \
             tc.tile_pool(name="mpool", bufs=2) as mpool, \
             tc.tile_pool(name="dpool", bufs=2) as dpool, \
             tc.tile_pool(name="hsb", bufs=3) as hsbp, \
             tc.tile_pool(name="ps", bufs=4, space="PSUM") as ps_pool:

            s_sb = pp.tile([128, S_FREE], dt.float32r)
            s8 = pp.tile([128, S_FREE], dt.float8e4)
            tapw8 = pp.tile([128, NDR * 256], dt.float8e4)
            w2b = pp.tile([128, 2 * 120], dt.float32r)
            b2r = pp.tile([128, 1], dt.float32)
            b1v = pp.tile([128, 1], dt.float32)

            # state streamed in four pieces, each fp8-cast as it lands, so
            # the first matmuls start ~2us earlier; weights go after the
            # first piece (needed by the first matmul group too)
            _cuts = [0, FRAME_OFF + 8 * FW, FRAME_OFF + 14 * FW,
                     FRAME_OFF + 20 * FW, S_FREE]
            nc.sync.dma_start(s_sb[:, _cuts[0]:_cuts[1]],
                              s_d[:, _cuts[0]:_cuts[1]])
            nc.sync.dma_start(tapw8[:], tapw_d[:])
            nc.sync.dma_start(w2b[:], w2b_d[:])
            nc.sync.dma_start(b2r[:], b2r_d[:])
            nc.sync.dma_start(b1v[:], b1_d[:])
            for _i in range(1, 4):
                nc.sync.dma_start(s_sb[:, _cuts[_i]:_cuts[_i + 1]],
                                  s_d[:, _cuts[_i]:_cuts[_i + 1]])
            # initial fp8 shadow (includes halo rows prepped by the host)
            for _i in range(4):
                nc.vector.tensor_copy(s8[:, _cuts[_i]:_cuts[_i + 1]],
                                      s_sb[:, _cuts[_i]:_cuts[_i + 1]])

            a0 = FRAME_OFF + FW + 1

            def emit_out(k):
                nc.sync.dma_start(
                    out_d[:, 6 * k * W:(6 * k + 6) * W].rearrange(
                        "p (r x) -> p r x", x=W),
                    s_sb[:, a0 + 780 * k:a0 + 780 * k + 6 * FW].rearrange(
                        "p (r x) -> p r x", x=FW)[:, :, 0:W].bitcast(
                            dt.float32),
                )

            def emit_l1(k, p):
                base = 64 * p
                hps = ps_pool.tile([128, 1024], dt.float32, tag="ps",
                                    name="hps_t")
                for cc in (1, 0):
                    c = 2 * k + cc
                    for i in range(NDR):
                        nc.tensor.matmul(
                            hps[:, 512 * cc:512 * cc + NCH],
                            tapw8[base:base + 56,
                                  256 * i:256 * i + 256].rearrange(
                                      "p (two m) -> p two m", two=2),
                            dr_rhs(s8, base, c, i),
                            start=(i == 0), stop=(i == NDR - 1),
                            tile_position=(base, 0),
                            perf_mode=DR,
                        )
                hsb = hsbp.tile([128, 2 * NCH], dt.float32r, tag=f"hsb{p}",
                                name="hsb_t")
                nc.scalar.activation(
                    hsb[:].rearrange("p (b x) -> p b x", x=NCH),
                    hps[:].rearrange("p (b x) -> p b x", b=2)[:, :, 0:NCH],
                    Relu, bias=b1v[:, 0:1], scale=1.0 / WSCALE,
                )
                return hsb

            def emit_l2_half(dps, hsb, p):
                # one slab-pair half of the L2 accumulation for both chunks;
                # each half is emitted one L1 group after its relu so the
                # dependency never blocks the in-order PE queue
                for cc in range(2):
                    nc.tensor.matmul(
                        dps[0:120, 512 * cc:512 * cc + NCH],
                        w2b[:, 120 * p:120 * p + 120],
                        hsb[:, NCH * cc:NCH * cc + NCH],
                        start=(p == 0), stop=(p == 1),
                    )

            def emit_stt(k, dps, m_sb, d_sb):
                # u = (delta + b2) * fire on DVE (PSUM src), into d_sb
                nc.vector.scalar_tensor_tensor(
                    d_sb[0:120, 780 * k:780 * k + 780].rearrange(
                        "p (b x) -> p b x", x=NCH),
                    dps[0:120].rearrange(
                        "p (b x) -> p b x", b=2)[:, :, 0:NCH],
                    b2r[0:120, 0:1],
                    m_sb[0:120, 780 * k:780 * k + 780].rearrange(
                        "p (b x) -> p b x", x=NCH),
                    A.add, A.mult,
                )

            def emit_add(k, eng, d_sb):
                a = FRAME_OFF + FW + 780 * k
                # plain TensorTensor Add: the only elementwise binop the
                # walrus gpsimd codegen accepts (STT is DVE/ACT-only)
                eng.tensor_add(
                    s_sb[0:120, a:a + 780],
                    s_sb[0:120, a:a + 780],
                    d_sb[0:120, 780 * k:780 * k + 780],
                )

            def emit_cast(k):
                a = FRAME_OFF + FW + 780 * k
                nc.vector.tensor_copy(s8[:, a:a + 780], s_sb[:, a:a + 780])

            def emit_shuffles(edge):
                if edge == 0:
                    # row 24 of quadrant g-1 -> frame row 0 of quadrant g
                    for g in range(1, 4):
                        nc.vector.stream_shuffle(
                            s8[32 * g:32 * g + 32,
                               FRAME_OFF:FRAME_OFF + FW],
                            s8[32 * (g - 1):32 * (g - 1) + 32,
                               FRAME_OFF + 24 * FW:FRAME_OFF + 25 * FW],
                            list(range(32)))
                else:
                    # row 1 of quadrant g+1 -> frame row 25 of quadrant g
                    for g in range(3):
                        nc.vector.stream_shuffle(
                            s8[32 * g:32 * g + 32,
                               FRAME_OFF + 25 * FW:FRAME_OFF + 26 * FW],
                            s8[32 * (g + 1):32 * (g + 1) + 32,
                               FRAME_OFF + FW:FRAME_OFF + 2 * FW],
                            list(range(32)))

            def emit_carry(carry):
                # finish the previous pair: second L2 half, mask, state add.
                # For the boundary pair (2) this runs at the START of the
                # NEXT step's emission, overlapped by its first L1 group.
                ck, cdps, chsb1, cm, cd = carry
                emit_l2_half(cdps, chsb1, 1)
                emit_stt(ck, cdps, cm, cd)
                if ck == 2:
                    emit_add(ck, nc.vector, cd)
                    emit_cast(ck)
                else:
                    emit_add(ck, nc.gpsimd, cd)

            # Constant pair order with pair 2 last: the next step's first
            # group mm(0) depends only on pairs {0,1,3} (all early), so pair
            # 2's whole tail — including its final L2 half — carries across
            # the step boundary and overlaps mm(0, t+1).
            korder = [0, 3, 1, 2]
            carry = None
            for it in range(steps * repeats):
                t = it % steps
                last = it == steps * repeats - 1
                m_sb = mpool.tile([128, COMP], dt.bfloat16, tag="m")
                nc.sync.dma_start(m_sb[:], m_d[t])
                d_sb = dpool.tile([128, COMP], dt.float32, tag="d")

                for k in korder:
                    hsb0 = emit_l1(k, 0)
                    if carry is not None:
                        emit_carry(carry)
                    hsb1 = emit_l1(k, 1)
                    dps = ps_pool.tile([128, 1024], dt.float32, tag="ps",
                                        name="dps_t")
                    emit_l2_half(dps, hsb0, 0)
                    carry = (k, dps, hsb1, m_sb, d_sb)

                if last:
                    # no next step: flush the carry per chunk so the final
                    # writeback overlaps the mask/add chain, skipping all
                    # fp8-shadow maintenance
                    for kk in (0, 3, 1):
                        emit_out(kk)
                    ck, cdps, chsb1, cm, cd = carry
                    carry = None
                    emit_l2_half(cdps, chsb1, 1)
                    for cc in range(2):
                        nc.vector.scalar_tensor_tensor(
                            cd[0:120, 780 * ck + NCH * cc:
                               780 * ck + NCH * cc + NCH],
                            cdps[0:120, 512 * cc:512 * cc + NCH],
                            b2r[0:120, 0:1],
                            cm[0:120, 780 * ck + NCH * cc:
                               780 * ck + NCH * cc + NCH],
                            A.add, A.mult,
                        )
                        a = FRAME_OFF + FW + 780 * ck + NCH * cc
                        # chunk 0's add on the idle Pool engine so the two
                        # chunks' flush chains run in parallel
                        (nc.gpsimd if cc == 0 else nc.vector).tensor_add(
                            s_sb[0:120, a:a + NCH],
                            s_sb[0:120, a:a + NCH],
                            cd[0:120, 780 * ck + NCH * cc:
                               780 * ck + NCH * cc + NCH],
                        )
                        nc.sync.dma_start(
                            out_d[:, (6 * ck + 3 * cc) * W:
                                  (6 * ck + 3 * cc + 3) * W].rearrange(
                                "p (r x) -> p r x", x=W),
                            s_sb[:, a0 + 780 * ck + NCH * cc:
                                 a0 + 780 * ck + NCH * cc + 3 * FW].rearrange(
                                "p (r x) -> p r x", x=FW)[:, :, 0:W].bitcast(
                                    dt.float32),
                        )
                else:
                    # s8 maintenance after the step's matmuls: later pairs'
                    # matmuls must read the pre-update boundary rows, and an
                    # early cast would stall them on a RAW hazard
                    emit_cast(3)
                    emit_shuffles(0)
                    emit_cast(0)
                    emit_cast(1)
                    emit_shuffles(25)

    nc.compile()
    return nc


def _prep_weights(w1, b1, w2, b2):
    sx = np.array([[-1, 0, 1], [-2, 0, 2], [-1, 0, 1]], np.float32) / 8.0
    sy = sx.T.copy()
    ident = np.zeros((3, 3), np.float32)
    ident[1, 1] = 1.0
    # Weff[o, c, dy, dx]
    weff = (np.einsum("oc,yx->ocyx", w1[:, 0::3], ident)
            + np.einsum("oc,yx->ocyx", w1[:, 1::3], sx)
            + np.einsum("oc,yx->ocyx", w1[:, 2::3], sy)).astype(np.float32)

    # DoubleRow pair lhsT: for pair-matmul i, [56 x 2 x 128] fp8 blocks at
    # partition bases 0 and 64 (one per slab pair). Rows 0-23 (even slab ch)
    # feed out cols 0-63; rows 32-55 (odd slab ch) feed cols 64-127.
    tapw8 = np.zeros((128, NDR * 256), F8)
    for i, pair in enumerate(TAP_PAIRS):
        for plane, ((dy, dx), level) in enumerate(pair):
            wt = weff[:, :, dy, dx].T * WSCALE   # [24, 64]
            w_hi = wt.astype(F8)
            if level == "hi":
                w8 = w_hi
            else:
                w8 = (wt - w_hi.astype(np.float32)).astype(F8)
            col = 256 * i + 128 * plane
            for pb in (0, 64):
                tapw8[pb:pb + 24, col:col + 64] = w8
                tapw8[pb + 32:pb + 56, col + 64:col + 128] = w8

    # layer2 lhsT per pair: K=128 (both h halves), M=120 with 24-col blocks
    # placing each slab's delta on its partition quadrant.
    w2b = np.zeros((128, 2 * 120), np.float32)
    for p in range(2):
        ge, go = 2 * p, 2 * p + 1
        w2b[0:64, 120 * p + 32 * ge:120 * p + 32 * ge + 24] = w2.T
        w2b[64:128, 120 * p + 32 * go:120 * p + 32 * go + 24] = w2.T

    b2r = np.zeros((128, 1), np.float32)
    b1v = np.zeros((128, 1), np.float32)
    for g in range(4):
        b2r[32 * g:32 * g + 24, 0] = b2
    b1v[0:64, 0] = b1
    b1v[64:128, 0] = b1
    return tapw8, w2b, b2r, b1v


def _prep_state(state):
    """state (B, C, H, W) -> per-core [128, S_FREE] framed slabs."""
    bufs = []
    for core in range(N_CORES):
        b = core // 2
        top = (core % 2) == 0
        r0 = 0 if top else H - SH
        buf = np.zeros((128, S_FREE), np.float32)
        for ch in range(C):
            full = np.zeros((SH + 2, FW), np.float32)
            full[1:SH + 1, 1:W + 1] = state[b, ch, r0:r0 + SH, :]
            if r0 > 0:
                full[0, 1:W + 1] = state[b, ch, r0 - 1, :]
            if r0 + SH < H:
                full[SH + 1, 1:W + 1] = state[b, ch, r0 + SH, :]
            for g in range(4):
                fr = full[g * SR:g * SR + FR, :]
                buf[32 * g + ch, FRAME_OFF:FRAME_OFF + FRAME] = fr.reshape(-1)
        bufs.append(buf)
    return bufs


def _prep_fire(masks):
    """masks (S, B, 1, H, W) -> per-core [S, 128, COMP] bf16 fire=(m<0.5),
    zero-padded so pad lanes/columns never update the state."""
    S = masks.shape[0]
    fire = (masks < FIRE_RATE).astype(np.float32)
    bufs = []
    for core in range(N_CORES):
        b = core // 2
        top = (core % 2) == 0
        r0 = 0 if top else H - SH
        mb = np.zeros((S, 128, COMP), np.float32)
        mrows = np.zeros((S, SH, FW), np.float32)
        mrows[:, :, 1:W + 1] = fire[:, b, 0, r0:r0 + SH, :]
        for g in range(4):
            seg = mrows[:, g * SR:(g + 1) * SR, :].reshape(S, COMP)
            mb[:, 32 * g:32 * g + C, :] = seg[:, None, :]
        bufs.append(mb.astype(BF16))
    return bufs


def prepare(inputs):
    """Build (nc, in_maps) for the given full inputs."""
    state = np.asarray(inputs["state"])
    w1, b1 = np.asarray(inputs["w1"]), np.asarray(inputs["b1"])
    w2, b2 = np.asarray(inputs["w2"]), np.asarray(inputs["b2"])
    masks = np.asarray(inputs["masks"])
    import os as _os
    steps = masks.shape[0]
    repeats = int(_os.environ.get("NCA_REPEAT", "1"))
    key = ("prog", steps, repeats)
    if key not in _cache:
        _cache[key] = _build_program(steps, repeats)
    nc = _cache[key]

    tapw8, w2b, b2r, b1v = _prep_weights(w1, b1, w2, b2)
    s_bufs = _prep_state(state)
    m_bufs = _prep_fire(masks)

    in_maps = []
    for core in range(N_CORES):
        in_maps.append({
            "s0": s_bufs[core],
            "fire": m_bufs[core],
            "tapw8": tapw8,
            "w2b": w2b,
            "b2r": b2r,
            "b1v": b1v,
        })
    return nc, in_maps


def unpack(results):
    out = np.zeros((B, C, H, W), np.float32)
    for core in range(N_CORES):
        o = results[core]["out"]  # [128, SR*W]
        b = core // 2
        top = (core % 2) == 0
        r0 = 0 if top else H - SH
        own0 = 0 if top else H // 2
        for g in range(4):
            rows = o[32 * g:32 * g + 24].reshape(C, SR, W)
            g0 = r0 + g * SR
            lo = max(g0, own0)
            hi = min(g0 + SR, own0 + H // 2)
            if lo < hi:
                out[b, :, lo:hi, :] = rows[:, lo - g0:hi - g0, :]
    return out


def kernel(state, w1, b1, w2, b2, masks):
    nc, in_maps = prepare({"state": state, "w1": w1, "b1": b1,
                           "w2": w2, "b2": b2, "masks": masks})

    from concourse.bass_utils import run_bass_kernel_spmd

    import os
    trace = bool(os.environ.get("NCA_TRACE"))
    kw = {}
    if trace:
        kw["trace"] = True
        if os.environ.get("NCA_TRACE_DIR"):
            kw["tmpdir"] = os.environ["NCA_TRACE_DIR"]
    res = run_bass_kernel_spmd(nc, in_maps, list(range(N_CORES)), **kw)
    global LAST_EXEC_NS
    LAST_EXEC_NS = res.exec_time_ns

    return unpack(res.results)
